# revision 1
# baseline (speedup 1.0000x reference)
"""Trainium2 Bass kernel for ConvScoreSSREM loss.

Computes, for B=16384 rows (data-parallel, 2048 rows per NeuronCore x 8):
    cm        = contexts @ mat_M                    [B, E]
    scores_k  = sum_e cm[b,e] * res_k[b,e]          k in 0..4
    out[b]    = log_softmax(scores)[:, 0]

Per-core plan (2048 rows, E=1024):
  - mat_M resident in SBUF as fp32r chunks (full-rate PE operand), staged
    through a small fp32 buffer.
  - row-tiles processed in pairs (256 rows): 1MB DMAs, split across the two
    HWDGE rings (sync + scalar) for parallel descriptor generation.
  - per 128-row tile: PE-transpose contexts (fp32, exact) to [e,b]; 16 fp32r
    matmuls accumulate cm[128b,1024e'] in PSUM; ACT copies cm to SBUF (frees
    the PSUM bank early, avoids DVE's PSUM access latency); 5 fused DVE
    multiply+reduce (scalar_tensor_tensor) ops produce the scores.
  - one log-softmax tail over the [128, 16, 5] score tile, single DMA out.
"""

import numpy as np

import concourse.bacc as bacc
import concourse.mybir as mybir
import concourse.tile as tile
from concourse import masks
from concourse.bass_utils import run_bass_kernel_spmd

B = 16384
E = 1024
NCORES = 8
BS = B // NCORES  # 2048 rows per core
P = 128
NT = BS // P      # 16 row-tiles per core
NG = NT // 2      # 8 pair-groups
KC = E // P       # 8 contraction chunks
NK = 5            # number of res tensors
NHALF = 512       # matmul moving free-dim (one PSUM bank of fp32)

F32 = mybir.dt.float32
F32R = mybir.dt.float32r

RES_NAMES = ["res0", "res1", "res2", "res3", "res4"]


DEFAULT_OPTS = dict(
    pair=True,         # 1MB pair loads vs 0.5MB single-tile loads
    split_rings=True,  # issue loads on both HWDGE rings (sync + scalar)
    group_copies=True, # drain 4 transposes per ACT copy vs 1
    use_cms=True,      # ACT-copy cm PSUM->SBUF before the DVE score ops
                       # (frees PSUM sooner; avoids DVE PSUM access latency)
    pcm_bufs=3,
    ptr_bufs=2,
    res_bufs=12,
    ctx_bufs=3,
    cms_bufs=3,
    ctx_split=False,     # load ctx per 128-row tile (0.5MB) even when pair=True
    m_pair=False,        # load mat_M in 1MB pair chunks (4 DMAs)
    m_after_first=False, # emit group-0 input loads before the mat_M loads
    h_outer=False,       # run each 512-wide PSUM bank's matmuls to completion
                         # before the other so DVE can start on bank 0 early
    junk_bufs=2,
)


def build_nc(repeat=1, internal_inputs=False, opts=None):
    """Build + compile the single-core Bass program (same program on all 8 cores).

    repeat>1 replays the steady-state compute loop; internal_inputs=True reads
    contexts/res from internal DRAM scratch instead of ExternalInputs (both are
    timing aids only)."""
    nc = bacc.Bacc("TRN2", debug=False, enable_asserts=False, num_devices=NCORES)

    if internal_inputs:
        ctx_d = nc.dram_tensor("contexts_i", (BS, E), F32, kind="Internal")
        res_d = [nc.dram_tensor(n + "_i", (BS, E), F32, kind="Internal") for n in RES_NAMES]
    else:
        ctx_d = nc.dram_tensor("contexts", (BS, E), F32, kind="ExternalInput")
        res_d = [nc.dram_tensor(n, (BS, E), F32, kind="ExternalInput") for n in RES_NAMES]
    m_d = nc.dram_tensor("mat_M", (E, E), F32, kind="ExternalInput")
    out_d = nc.dram_tensor("out", (BS,), F32, kind="ExternalOutput")

    o = dict(DEFAULT_OPTS)
    if opts:
        o.update(opts)
    with tile.TileContext(nc) as tc:
        _body(nc, tc, ctx_d.ap(), [r.ap() for r in res_d], m_d.ap(), out_d.ap(),
              repeat=repeat, o=o)

    nc.compile()
    return nc


def _body(nc, tc, ctx_d, res_d, m_d, out_d, repeat=1, o=None):
    o = o or DEFAULT_OPTS
    na = 2 if o["pair"] else 1
    ng = NT // na
    # DRAM views with groups split out: row (g*na + a)*128 + p.
    # DMA pairs source/dest elements in flat AP order, so the DRAM view must
    # match the SBUF tile's [p, a, e] dim order.
    ctx_g = ctx_d.rearrange("(g a p) e -> g p a e", a=na, p=P)
    res_g = [r.rearrange("(g a p) e -> g p a e", a=na, p=P) for r in res_d]

    with (
        tc.tile_pool(name="mpool", bufs=1) as mpool,
        tc.tile_pool(name="mstage", bufs=2) as mstage,
        tc.tile_pool(name="ctxp", bufs=o["ctx_bufs"]) as ctxp,
        tc.tile_pool(name="resp", bufs=o["res_bufs"]) as resp,
        tc.tile_pool(name="ctxTp", bufs=2) as ctxTp,
        tc.tile_pool(name="cmsb", bufs=o["cms_bufs"]) as cmsb,
        tc.tile_pool(name="junkp", bufs=o["junk_bufs"]) as junkp,
        tc.tile_pool(name="smallp", bufs=1) as smallp,
        tc.tile_pool(name="pcm", bufs=o["pcm_bufs"], space="PSUM") as pcm,
        tc.tile_pool(name="ptr", bufs=o["ptr_bufs"], space="PSUM") as ptr,
    ):
        ident = smallp.tile([P, P], F32)
        masks.make_identity(nc, ident[:])

        # mat_M resident in fp32r: m_sbr[p, k, :] = M[k*128 + p, :]
        m_sbr = mpool.tile([P, KC, E], F32R)
        m_pair_v = m_d.rearrange("(q a p) e -> q p a e", a=2, p=P)

        def load_m():
            if o["m_pair"]:
                for q in range(KC // 2):
                    stg = mstage.tile([P, 2, E], F32, tag="mstg")
                    nc.sync.dma_start(stg[:], m_pair_v[q])
                    nc.scalar.copy(
                        m_sbr[:, 2 * q : 2 * q + 2, :].rearrange("p a e -> p (a e)"),
                        stg[:].rearrange("p a e -> p (a e)"),
                    )
            else:
                for k in range(KC):
                    stg = mstage.tile([P, E], F32, tag="mstg")
                    nc.sync.dma_start(stg[:], m_d[k * P : (k + 1) * P, :])
                    nc.scalar.copy(m_sbr[:, k, :], stg[:])

        if not o["m_after_first"]:
            load_m()

        scores = smallp.tile([P, NT, NK], F32)

        def load_group(g):
            if o["ctx_split"] and na == 2:
                ctx_t = ctxp.tile([P, na, E], F32, tag="ctx")
                for a in range(na):
                    (nc.scalar if o["split_rings"] else nc.sync).dma_start(
                        ctx_t[:, a, :], ctx_g[g, :, a, :]
                    )
            else:
                ctx_t = ctxp.tile([P, na, E], F32, tag="ctx")
                (nc.scalar if o["split_rings"] else nc.sync).dma_start(ctx_t[:], ctx_g[g])
            res_t = []
            for k in range(NK):
                r = resp.tile([P, na, E], F32, tag="res")
                eng = nc.sync if (k < 3 or not o["split_rings"]) else nc.scalar
                eng.dma_start(r[:], res_g[k][g])
                res_t.append(r)
            return ctx_t, res_t

        for _rep in range(repeat):
            for g in range(ng):
                ctx_t, res_t = load_group(g)
                if _rep == 0 and g == 0 and o["m_after_first"]:
                    load_m()

                for a in range(na):
                    t = na * g + a
                    # transpose contexts tile: ctxT[:, k, :] = ctx[:, k*128:...].T
                    ctxT = ctxTp.tile([P, KC, P], F32R, tag="ctxT")
                    if o["group_copies"]:
                        # 4 transposes land in one PSUM bank ([128,512]); one
                        # ACT copy drains each bank
                        for q in range(2):
                            pt = ptr.tile([P, 4, P], F32, tag="pt")
                            for j in range(4):
                                k = 4 * q + j
                                nc.tensor.transpose(
                                    pt[:, j, :], ctx_t[:, a, k * P : (k + 1) * P], ident[:]
                                )
                            nc.scalar.copy(ctxT[:, 4 * q : 4 * q + 4, :], pt[:])
                    else:
                        for k in range(KC):
                            pt = ptr.tile([P, P], F32, tag="pt")
                            nc.tensor.transpose(
                                pt[:], ctx_t[:, a, k * P : (k + 1) * P], ident[:]
                            )
                            nc.scalar.copy(ctxT[:, k, :], pt[:])

                    # cm[128b, 1024e'] accumulated over 8 contraction chunks
                    cm = pcm.tile([P, E], F32, tag="cm")
                    hk = (
                        [(h, k) for h in range(2) for k in range(KC)]
                        if o["h_outer"]
                        else [(h, k) for k in range(KC) for h in range(2)]
                    )
                    for h, k in hk:
                        nc.tensor.matmul(
                            cm[:, h * NHALF : (h + 1) * NHALF],
                            ctxT[:, k, :],
                            m_sbr[:, k, h * NHALF : (h + 1) * NHALF],
                            start=(k == 0),
                            stop=(k == KC - 1),
                        )

                    if o["use_cms"]:
                        # PSUM -> SBUF decouples the PSUM bank from the DVE reads
                        cm_s = cmsb.tile([P, E], F32, tag="cms")
                        nc.scalar.copy(cm_s[:], cm[:])
                    else:
                        cm_s = cm

                    # scores[:, t, k] = sum_e' cm * res_k (fused mul+accum on DVE)
                    for k in range(NK):
                        junk = junkp.tile([P, E], F32, tag="junk")
                        nc.vector.scalar_tensor_tensor(
                            out=junk[:],
                            in0=cm_s[:],
                            scalar=1.0,
                            in1=res_t[k][:, a, :],
                            op0=mybir.AluOpType.mult,
                            op1=mybir.AluOpType.mult,
                            accum_out=scores[:, t, k : k + 1],
                        )

        # ---- log-softmax tail over [P, NT, NK] ----
        mx = smallp.tile([P, NT], F32)
        nc.vector.tensor_reduce(
            out=mx[:], in_=scores[:], axis=mybir.AxisListType.X, op=mybir.AluOpType.max
        )
        d = smallp.tile([P, NT, NK], F32)
        mx_b = mx[:, :, None].broadcast_to([P, NT, NK])
        nc.vector.tensor_tensor(
            out=d[:], in0=scores[:], in1=mx_b, op=mybir.AluOpType.subtract
        )
        ex = smallp.tile([P, NT, NK], F32)
        nc.scalar.activation(ex[:], d[:], mybir.ActivationFunctionType.Exp)
        ssum = smallp.tile([P, NT], F32)
        nc.vector.tensor_reduce(
            out=ssum[:], in_=ex[:], axis=mybir.AxisListType.X, op=mybir.AluOpType.add
        )
        lse = smallp.tile([P, NT], F32)
        nc.scalar.activation(lse[:], ssum[:], mybir.ActivationFunctionType.Ln)
        outsb = smallp.tile([P, NT], F32)
        nc.vector.tensor_sub(outsb[:], d[:, :, 0], lse[:])

        nc.sync.dma_start(out_d.rearrange("(t p) -> p t", p=P), outsb[:])


_NC_CACHE = None


def _get_nc():
    global _NC_CACHE
    if _NC_CACHE is None:
        _NC_CACHE = build_nc()
    return _NC_CACHE


def make_in_maps(contexts, res_pos, res_neg1, res_neg2, res_neg3, res_neg4, mat_M):
    contexts = np.asarray(contexts, dtype=np.float32)
    ress = [
        np.asarray(r, dtype=np.float32)
        for r in (res_pos, res_neg1, res_neg2, res_neg3, res_neg4)
    ]
    mat_M = np.asarray(mat_M, dtype=np.float32)
    in_maps = []
    for c in range(NCORES):
        sl = slice(c * BS, (c + 1) * BS)
        m = {"contexts": contexts[sl], "mat_M": mat_M}
        for name, r in zip(RES_NAMES, ress):
            m[name] = r[sl]
        in_maps.append(m)
    return in_maps


def kernel(contexts, res_pos, res_neg1, res_neg2, res_neg3, res_neg4, mat_M):
    nc = _get_nc()
    in_maps = make_in_maps(
        contexts, res_pos, res_neg1, res_neg2, res_neg3, res_neg4, mat_M
    )
    res = run_bass_kernel_spmd(nc, in_maps, core_ids=list(range(NCORES)))
    out = np.concatenate([res.results[c]["out"] for c in range(NCORES)])
    return out.astype(np.float32, copy=False)



# revision 6
# speedup vs baseline: 43.6349x; 43.6349x over previous
"""Trainium2 Bass kernel for ConvScoreSSREM loss.

Computes, for B=16384 rows (data-parallel, 2048 rows per NeuronCore x 8):
    cm        = contexts @ mat_M                    [B, E]
    scores_k  = sum_e cm[b,e] * res_k[b,e]          k in 0..4
    out[b]    = log_softmax(scores)[:, 0]

Per-core plan (2048 rows, E=1024):
  - mat_M resident in SBUF as fp32r chunks (full-rate PE operand), staged
    through a small fp32 buffer.
  - row-tiles processed in pairs (256 rows): 1MB DMAs, split across the two
    HWDGE rings (sync + scalar) for parallel descriptor generation.
  - per 128-row tile: PE-transpose contexts (fp32, exact) to [e,b]; 16 fp32r
    matmuls accumulate cm[128b,1024e'] in PSUM; ACT copies cm to SBUF (frees
    the PSUM bank early, avoids DVE's PSUM access latency); 5 fused DVE
    multiply+reduce (scalar_tensor_tensor) ops produce the scores.
  - one log-softmax tail over the [128, 16, 5] score tile, single DMA out.
"""

import numpy as np

import concourse.bacc as bacc
import concourse.mybir as mybir
import concourse.tile as tile
from concourse import masks
from concourse.bass_utils import run_bass_kernel_spmd

B = 16384
E = 1024
NCORES = 8
BS = B // NCORES  # 2048 rows per core
P = 128
NT = BS // P      # 16 row-tiles per core
NG = NT // 2      # 8 pair-groups
KC = E // P       # 8 contraction chunks
NK = 5            # number of res tensors
NHALF = 512       # matmul moving free-dim (one PSUM bank of fp32)

F32 = mybir.dt.float32
F32R = mybir.dt.float32r

RES_NAMES = ["res0", "res1", "res2", "res3", "res4"]


DEFAULT_OPTS = dict(
    pair=True,         # 1MB pair loads vs 0.5MB single-tile loads
    split_rings=True,  # issue loads on both HWDGE rings (sync + scalar)
    group_copies=True, # drain 4 transposes per ACT copy vs 1
    use_cms=True,      # ACT-copy cm PSUM->SBUF before the DVE score ops
                       # (frees PSUM sooner; avoids DVE PSUM access latency)
    pcm_bufs=3,
    ptr_bufs=2,
    res_bufs=12,
    ctx_bufs=3,
    cms_bufs=3,
    ctx_split=False,     # load ctx per 128-row tile (0.5MB) even when pair=True
    m_pair=False,        # load mat_M in 1MB pair chunks (4 DMAs)
    m_after_first=False, # emit group-0 input loads before the mat_M loads
    h_outer=False,       # run each 512-wide PSUM bank's matmuls to completion
                         # before the other so DVE can start on bank 0 early
    junk_bufs=2,
    # 3-queue DMA spread: per-group queue for (ctx, res0..res4); 'pool' uses
    # the GpSimd SWDGE ring, a third descriptor-gen path in parallel with the
    # two HWDGE rings (sync + scalar). Even/odd group patterns balance
    # SP 16 / ACT 12 / Pool 20 of the 48 1MB loads per pass.
    qmap_even=("sp", "pool", "sp", "pool", "act", "sp"),
    qmap_odd=("sp", "pool", "sp", "pool", "act", "pool"),
    pool_scores_k=None,  # Pool TensorScalarPtr does not lower through walrus
                         # (NEFF compile crash) — keep all score ops on DVE
)


def build_nc(repeat=1, internal_inputs=False, opts=None):
    """Build + compile the single-core Bass program (same program on all 8 cores).

    repeat>1 replays the steady-state compute loop; internal_inputs=True reads
    contexts/res from internal DRAM scratch instead of ExternalInputs (both are
    timing aids only)."""
    nc = bacc.Bacc("TRN2", debug=False, enable_asserts=False, num_devices=NCORES)

    if internal_inputs:
        ctx_d = nc.dram_tensor("contexts_i", (BS, E), F32, kind="Internal")
        res_d = [nc.dram_tensor(n + "_i", (BS, E), F32, kind="Internal") for n in RES_NAMES]
    else:
        ctx_d = nc.dram_tensor("contexts", (BS, E), F32, kind="ExternalInput")
        res_d = [nc.dram_tensor(n, (BS, E), F32, kind="ExternalInput") for n in RES_NAMES]
    m_d = nc.dram_tensor("mat_M", (E, E), F32, kind="ExternalInput")
    out_d = nc.dram_tensor("out", (BS,), F32, kind="ExternalOutput")

    o = dict(DEFAULT_OPTS)
    if opts:
        o.update(opts)
    with tile.TileContext(nc) as tc:
        _body(nc, tc, ctx_d.ap(), [r.ap() for r in res_d], m_d.ap(), out_d.ap(),
              repeat=repeat, o=o)

    nc.compile()
    return nc


def _body(nc, tc, ctx_d, res_d, m_d, out_d, repeat=1, o=None):
    o = o or DEFAULT_OPTS
    na = 2 if o["pair"] else 1
    ng = NT // na
    # DRAM views with groups split out: row (g*na + a)*128 + p.
    # DMA pairs source/dest elements in flat AP order, so the DRAM view must
    # match the SBUF tile's [p, a, e] dim order.
    ctx_g = ctx_d.rearrange("(g a p) e -> g p a e", a=na, p=P)
    res_g = [r.rearrange("(g a p) e -> g p a e", a=na, p=P) for r in res_d]

    with (
        tc.tile_pool(name="mpool", bufs=1) as mpool,
        tc.tile_pool(name="mstage", bufs=2) as mstage,
        tc.tile_pool(name="ctxp", bufs=o["ctx_bufs"]) as ctxp,
        tc.tile_pool(name="resp", bufs=o["res_bufs"]) as resp,
        tc.tile_pool(name="ctxTp", bufs=2) as ctxTp,
        tc.tile_pool(name="cmsb", bufs=o["cms_bufs"]) as cmsb,
        tc.tile_pool(name="junkp", bufs=o["junk_bufs"]) as junkp,
        tc.tile_pool(name="smallp", bufs=1) as smallp,
        tc.tile_pool(name="pcm", bufs=o["pcm_bufs"], space="PSUM") as pcm,
        tc.tile_pool(name="ptr", bufs=o["ptr_bufs"], space="PSUM") as ptr,
    ):
        ident = smallp.tile([P, P], F32)
        masks.make_identity(nc, ident[:])

        # mat_M resident in fp32r: m_sbr[p, k, :] = M[k*128 + p, :]
        m_sbr = mpool.tile([P, KC, E], F32R)
        m_pair_v = m_d.rearrange("(q a p) e -> q p a e", a=2, p=P)

        def load_m():
            if o["m_pair"]:
                for q in range(KC // 2):
                    stg = mstage.tile([P, 2, E], F32, tag="mstg")
                    nc.sync.dma_start(stg[:], m_pair_v[q])
                    nc.scalar.copy(
                        m_sbr[:, 2 * q : 2 * q + 2, :].rearrange("p a e -> p (a e)"),
                        stg[:].rearrange("p a e -> p (a e)"),
                    )
            else:
                for k in range(KC):
                    stg = mstage.tile([P, E], F32, tag="mstg")
                    nc.sync.dma_start(stg[:], m_d[k * P : (k + 1) * P, :])
                    nc.scalar.copy(m_sbr[:, k, :], stg[:])

        if not o["m_after_first"]:
            load_m()

        scores = smallp.tile([P, NT, NK], F32)

        ENG = {"sp": nc.sync, "act": nc.scalar, "pool": nc.gpsimd}

        def load_group(g):
            qmap = o["qmap_even"] if g % 2 == 0 else o["qmap_odd"]
            ctx_t = ctxp.tile([P, na, E], F32, tag="ctx")
            ENG[qmap[0]].dma_start(ctx_t[:], ctx_g[g])
            res_t = []
            for k in range(NK):
                r = resp.tile([P, na, E], F32, tag="res")
                ENG[qmap[k + 1]].dma_start(r[:], res_g[k][g])
                res_t.append(r)
            return ctx_t, res_t

        for _rep in range(repeat):
            for g in range(ng):
                ctx_t, res_t = load_group(g)
                if _rep == 0 and g == 0 and o["m_after_first"]:
                    load_m()

                for a in range(na):
                    t = na * g + a
                    # transpose contexts tile: ctxT[:, k, :] = ctx[:, k*128:...].T
                    ctxT = ctxTp.tile([P, KC, P], F32R, tag="ctxT")
                    if o["group_copies"]:
                        # 4 transposes land in one PSUM bank ([128,512]); one
                        # ACT copy drains each bank
                        for q in range(2):
                            pt = ptr.tile([P, 4, P], F32, tag="pt")
                            for j in range(4):
                                k = 4 * q + j
                                nc.tensor.transpose(
                                    pt[:, j, :], ctx_t[:, a, k * P : (k + 1) * P], ident[:]
                                )
                            nc.scalar.copy(ctxT[:, 4 * q : 4 * q + 4, :], pt[:])
                    else:
                        for k in range(KC):
                            pt = ptr.tile([P, P], F32, tag="pt")
                            nc.tensor.transpose(
                                pt[:], ctx_t[:, a, k * P : (k + 1) * P], ident[:]
                            )
                            nc.scalar.copy(ctxT[:, k, :], pt[:])

                    # cm[128b, 1024e'] accumulated over 8 contraction chunks
                    cm = pcm.tile([P, E], F32, tag="cm")
                    hk = (
                        [(h, k) for h in range(2) for k in range(KC)]
                        if o["h_outer"]
                        else [(h, k) for k in range(KC) for h in range(2)]
                    )
                    for h, k in hk:
                        nc.tensor.matmul(
                            cm[:, h * NHALF : (h + 1) * NHALF],
                            ctxT[:, k, :],
                            m_sbr[:, k, h * NHALF : (h + 1) * NHALF],
                            start=(k == 0),
                            stop=(k == KC - 1),
                        )

                    if o["use_cms"]:
                        # PSUM -> SBUF decouples the PSUM bank from the DVE reads
                        cm_s = cmsb.tile([P, E], F32, tag="cms")
                        nc.scalar.copy(cm_s[:], cm[:])
                    else:
                        cm_s = cm

                    # scores[:, t, k] = sum_e' cm * res_k (fused mul+accum on
                    # DVE; one op per even tile offloaded to Pool)
                    for k in range(NK):
                        junk = junkp.tile([P, E], F32, tag="junk")
                        eng = (
                            nc.gpsimd
                            if (k == o["pool_scores_k"] and t % 2 == 0)
                            else nc.vector
                        )
                        eng.scalar_tensor_tensor(
                            out=junk[:],
                            in0=cm_s[:],
                            scalar=1.0,
                            in1=res_t[k][:, a, :],
                            op0=mybir.AluOpType.mult,
                            op1=mybir.AluOpType.mult,
                            accum_out=scores[:, t, k : k + 1],
                        )

        # ---- log-softmax tail over [P, NT, NK] ----
        mx = smallp.tile([P, NT], F32)
        nc.vector.tensor_reduce(
            out=mx[:], in_=scores[:], axis=mybir.AxisListType.X, op=mybir.AluOpType.max
        )
        d = smallp.tile([P, NT, NK], F32)
        mx_b = mx[:, :, None].broadcast_to([P, NT, NK])
        nc.vector.tensor_tensor(
            out=d[:], in0=scores[:], in1=mx_b, op=mybir.AluOpType.subtract
        )
        ex = smallp.tile([P, NT, NK], F32)
        nc.scalar.activation(ex[:], d[:], mybir.ActivationFunctionType.Exp)
        ssum = smallp.tile([P, NT], F32)
        nc.vector.tensor_reduce(
            out=ssum[:], in_=ex[:], axis=mybir.AxisListType.X, op=mybir.AluOpType.add
        )
        lse = smallp.tile([P, NT], F32)
        nc.scalar.activation(lse[:], ssum[:], mybir.ActivationFunctionType.Ln)
        outsb = smallp.tile([P, NT], F32)
        nc.vector.tensor_sub(outsb[:], d[:, :, 0], lse[:])

        nc.sync.dma_start(out_d.rearrange("(t p) -> p t", p=P), outsb[:])


_NC_CACHE = None


def _get_nc():
    global _NC_CACHE
    if _NC_CACHE is None:
        _NC_CACHE = build_nc()
    return _NC_CACHE


def make_in_maps(contexts, res_pos, res_neg1, res_neg2, res_neg3, res_neg4, mat_M):
    contexts = np.asarray(contexts, dtype=np.float32)
    ress = [
        np.asarray(r, dtype=np.float32)
        for r in (res_pos, res_neg1, res_neg2, res_neg3, res_neg4)
    ]
    mat_M = np.asarray(mat_M, dtype=np.float32)
    in_maps = []
    for c in range(NCORES):
        sl = slice(c * BS, (c + 1) * BS)
        m = {"contexts": contexts[sl], "mat_M": mat_M}
        for name, r in zip(RES_NAMES, ress):
            m[name] = r[sl]
        in_maps.append(m)
    return in_maps


def kernel(contexts, res_pos, res_neg1, res_neg2, res_neg3, res_neg4, mat_M):
    nc = _get_nc()
    in_maps = make_in_maps(
        contexts, res_pos, res_neg1, res_neg2, res_neg3, res_neg4, mat_M
    )
    res = run_bass_kernel_spmd(nc, in_maps, core_ids=list(range(NCORES)))
    out = np.concatenate([res.results[c]["out"] for c in range(NCORES)])
    return out.astype(np.float32, copy=False)

